# revision 29
# baseline (speedup 1.0000x reference)
"""Trainium2 Bass kernel for nn_Classifier_86260123173820 (GNN message passing).

Strategy (8 NeuronCores, pure data parallelism, 8 graphs per core):
  - Per graph, the 4 message-passing layers apply a fixed sparse operator
    (segment_sum over edges + residual).  We fold the residual into a dense
    augmented adjacency  A_aug = A + I  (integer counts <= 16, exact in fp8)
    and evaluate  pooledT = splits(q)^T @ A_aug^T  on the PE, streaming the
    fp8 A_aug^T as the moving operand against a bf16 TRIPLE-split stationary
    (hi/lo/lo2 at columns 0/32/64), giving fp32-accurate products with fp32
    PSUM accumulation (the sort-pooling channel needs ~1e-7).
  - v2 layout: the pass runs kt-outer in four 512-column quarters so each
    quarter's epilogue (copy + partition-shifted adds + fused bias*invdeg +
    tanh) overlaps the next quarter's matmuls.  tanh results land directly
    in a per-graph f-major SBUF cat accumulator catT [97, 2048]; the next
    layer's q matmuls read their stationary straight out of catT (weights
    replicated at partition bases 0/32/64), and cat is transposed to DRAM
    node-major once per graph (16 PE transposes instead of 64).
  - e2n (edge_feat segment_sum) DMAs 4-chunk groups of the host-padded,
    row-sorted edge tensor, reduces once per group on DVE and transposes
    the [128,128] result in one PE op per group.
  - Sort-pooling top-30 per graph uses DVE max8/max_index/match_replace;
    the head gathers cat rows from DRAM by index and runs batched on PE.
"""

import numpy as np
import ml_dtypes

B, NPG, DEG = 64, 2048, 16
N, E = B * NPG, B * NPG * DEG
NF, EF = 128, 32
K = 30
TLD = 97
C1, C2, KW2 = 16, 32, 5
HID, NCLS = 128, 10
NCORES = 8
GPC = B // NCORES          # graphs per core
EPG = NPG * DEG            # edges per graph
NCH = NPG // 128           # 128-node chunks per graph (16)
NGRP = NCH // 4            # 4-chunk groups (4)

_cache = {}


def _build_program(S):
    import concourse.bass as bass
    import concourse.bacc as bacc
    import concourse.mybir as mybir
    import concourse.tile as tile
    from concourse.masks import make_identity
    from concourse.tile_rust import add_dep_helper
    dt = mybir.dt
    AF = mybir.ActivationFunctionType
    OP = mybir.AluOpType
    AX = mybir.AxisListType

    nc = bacc.Bacc("TRN2", target_bir_lowering=False, debug=False,
                   num_devices=NCORES)

    # ---- inputs (per core) ----
    d_nfT = nc.dram_tensor("nfT", [GPC, NF, NPG], dt.float32, kind="ExternalInput")
    d_efp = nc.dram_tensor("efp", [GPC, NGRP, 128, 4, EF, S], dt.float32, kind="ExternalInput")
    d_at8 = nc.dram_tensor("at8", [GPC, 128, NCH, NPG], dt.float8e4, kind="ExternalInput")
    d_degb = nc.dram_tensor("degb", [GPC, 32, NPG], dt.float32, kind="ExternalInput")
    d_w0a = nc.dram_tensor("w0aT", [NF, 32], dt.float32, kind="ExternalInput")
    d_w0b = nc.dram_tensor("w0bT", [EF, 32], dt.float32, kind="ExternalInput")
    d_w1 = nc.dram_tensor("w1T3", [96, 32], dt.float32, kind="ExternalInput")
    d_w2 = nc.dram_tensor("w2T3", [96, 32], dt.float32, kind="ExternalInput")
    d_w3 = nc.dram_tensor("w3T3", [96, 1], dt.float32, kind="ExternalInput")
    d_b0 = nc.dram_tensor("b0c", [32, 1], dt.float32, kind="ExternalInput")
    d_b1 = nc.dram_tensor("b1c", [32, 1], dt.float32, kind="ExternalInput")
    d_b2 = nc.dram_tensor("b2c", [32, 1], dt.float32, kind="ExternalInput")
    d_b3 = nc.dram_tensor("b3c", [1, 1], dt.float32, kind="ExternalInput")
    d_wc1 = nc.dram_tensor("wc1T", [TLD, C1], dt.float32, kind="ExternalInput")
    d_wc2 = nc.dram_tensor("wc2T", [C1, KW2, C2], dt.float32, kind="ExternalInput")
    d_bc1 = nc.dram_tensor("bc1c", [C1, 1], dt.float32, kind="ExternalInput")
    d_bc2 = nc.dram_tensor("bc2c", [C2, 1], dt.float32, kind="ExternalInput")
    d_wh = nc.dram_tensor("whT", [C2, 11, HID], dt.float32, kind="ExternalInput")
    d_bh = nc.dram_tensor("bhc", [HID, 1], dt.float32, kind="ExternalInput")
    d_wo = nc.dram_tensor("woT", [HID, NCLS], dt.float32, kind="ExternalInput")
    d_bo = nc.dram_tensor("boc", [NCLS, 1], dt.float32, kind="ExternalInput")

    d_e3 = nc.dram_tensor("e3c", [96, 32], dt.float32, kind="ExternalInput")

    d_cat = nc.dram_tensor("catd", [GPC * NPG, TLD], dt.float32)  # internal
    d_tis = nc.dram_tensor("tis", [GPC, 32], dt.uint32)  # internal scratch
    d_out = nc.dram_tensor("out", [GPC, NCLS], dt.float32, kind="ExternalOutput")

    LAYERS = [(d_w1, d_b1, 32), (d_w2, d_b2, 32), (d_w3, d_b3, 1)]

    with tile.TileContext(nc) as tc:
        with (
            tc.tile_pool(name="pw", bufs=1) as pw,          # persistent weights
            tc.tile_pool(name="pA", bufs=2) as pA,          # A^T fp8, per graph
            tc.tile_pool(name="pNF", bufs=2) as pNF,        # nfT/degb per graph
            tc.tile_pool(name="pEF", bufs=2) as pEF,        # efp DMA pieces
            tc.tile_pool(name="pCT", bufs=3) as pCT,        # catT f-major per graph
            tc.tile_pool(name="pG", bufs=2) as pG,          # catn (dma-out overlap)
            tc.tile_pool(name="pE2", bufs=2) as pE2,        # e2nT
            tc.tile_pool(name="pL", bufs=2) as pL,          # per-layer transients
            tc.tile_pool(name="pS", bufs=1) as pS,          # sortbuf & head
            tc.tile_pool(name="psQ", bufs=2, space="PSUM") as psQ,
            tc.tile_pool(name="psP", bufs=4, space="PSUM") as psP,
            tc.tile_pool(name="psT", bufs=2, space="PSUM") as psT,
        ):
            # persistent small tensors
            ident = pw.tile([128, 128], dt.float32)
            make_identity(nc, ident[:])
            w0a = pw.tile([NF, 32], dt.float32)
            w0b = pw.tile([EF, 32], dt.float32)
            nc.sync.dma_start(w0a[:], d_w0a.ap())
            nc.sync.dma_start(w0b[:], d_w0b.ap())
            wl = []
            for li, (dW, dB, w) in enumerate(LAYERS):
                tW = pw.tile([96, w], dt.float32, tag=f"lw{li}")
                tB = pw.tile([w, 1], dt.float32, tag=f"lb{li}")
                nc.sync.dma_start(tW[:], dW.ap())
                nc.sync.dma_start(tB[:], dB.ap())
                wl.append((tW, tB, w))
            b0 = pw.tile([32, 1], dt.float32)
            nc.sync.dma_start(b0[:], d_b0.ap())
            e3 = pw.tile([96, 32], dt.float32)
            nc.sync.dma_start(e3[:], d_e3.ap())
            wc1 = pw.tile([TLD, C1], dt.float32)
            wc2 = pw.tile([C1, KW2, C2], dt.float32)
            bc1 = pw.tile([C1, 1], dt.float32)
            bc2 = pw.tile([C2, 1], dt.float32)
            wh = pw.tile([C2, 11, HID], dt.float32)
            bh = pw.tile([HID, 1], dt.float32)
            wo = pw.tile([HID, NCLS], dt.float32)
            bo = pw.tile([NCLS, 1], dt.float32)
            for t, d in ((wc1, d_wc1), (wc2, d_wc2), (bc1, d_bc1), (bc2, d_bc2),
                         (wh, d_wh), (bh, d_bh), (wo, d_wo), (bo, d_bo)):
                nc.sync.dma_start(t[:], d.ap())

            sortbuf = pS.tile([GPC, NPG], dt.float32)
            cat_dmas = []

            def emit_inputs(g):
                at = pA.tile([128, NCH, NPG], dt.float8e4, tag="at")
                nc.sync.dma_start(at[:], d_at8.ap()[g])
                nfT = pNF.tile([NF, NPG], dt.float32, tag="nf")
                nc.sync.dma_start(nfT[:], d_nfT.ap()[g])
                degb = pNF.tile([32, NPG], dt.float32, tag="deg")
                nc.sync.dma_start(degb[:], d_degb.ap()[g])
                return at, nfT, degb

            def emit_cat_group(g, catT, grp):
                # cat -> node-major -> DRAM ([2048, 97]) for one 4-chunk group
                d_cat_g = d_cat.ap().rearrange(
                    "(g c p) d -> g p c d", g=GPC, p=128)[g]
                catn = pG.tile([128, 4, TLD], dt.float32, tag="cat")
                for c in range(4):
                    ch = grp * 4 + c
                    ptr = psT.tile([128, TLD], dt.float32, tag="tr")
                    nc.tensor.transpose(
                        ptr[:], catT[:, ch * 128:(ch + 1) * 128],
                        ident[0:TLD, 0:TLD])
                    nc.vector.tensor_copy(catn[:, c, :], ptr[:])
                cat_dmas.append(nc.scalar.dma_start(
                    d_cat_g[:, grp * 4:grp * 4 + 4], catn[:]).ins)

            def emit_e2n_group(g, e2nT, grp):
                # 2-chunk DMA pieces, one 4-chunk reduce group + PE transpose
                red4 = pL.tile([128, 4, EF], dt.float32, tag="red")
                for hh in range(2):
                    ef2 = pEF.tile([128, 2, EF, S], dt.float32, tag="ef")
                    nc.sync.dma_start(ef2[:], d_efp.ap()[g, grp, :, hh * 2:hh * 2 + 2])
                    nc.vector.tensor_reduce(
                        red4[:, hh * 2:hh * 2 + 2, :], ef2[:], axis=AX.X, op=OP.add)
                ptr = psT.tile([128, 128], dt.float32, tag="tr")
                nc.tensor.transpose(
                    ptr[:], red4[:].rearrange("p c f -> p (c f)"), ident[:])
                for c in range(4):
                    ch = grp * 4 + c
                    nc.vector.tensor_copy(
                        e2nT[:, ch * 128:(ch + 1) * 128],
                        ptr[c * 32:(c + 1) * 32, :])

            # pipeline prologue: graph 0 inputs + e2n emitted up front; each
            # later graph's input phase is emitted interleaved with the
            # PREVIOUS graph's layer stack so the DVE queue never puts the
            # next graph's reduces behind this graph's last epilogue.
            gtiles = {0: emit_inputs(0)}
            e2nT_first = pE2.tile([EF, NPG], dt.float32, tag="e2nT")
            e2nTs = {0: e2nT_first}
            for grp in range(NGRP):
                emit_e2n_group(0, e2nTs[0], grp)

            for g in range(GPC):
                at, nfT, degb = gtiles.pop(g)
                e2nT = e2nTs.pop(g)
                if g + 1 < GPC:
                    gtiles[g + 1] = emit_inputs(g + 1)
                    e2nT_next = pE2.tile([EF, NPG], dt.float32, tag="e2nT")
                    e2nTs[g + 1] = e2nT_next

                # ---- catT: f-major cat accumulator for this graph ----
                catT = pCT.tile([TLD, NPG], dt.float32, tag="catT")

                bias = b0
                for li in range(4):
                    w = 32 if li < 3 else 1
                    # -- q (node-major, psum [128, NCH*w]) --
                    qnm = psQ.tile([128, NCH * w], dt.float32, tag="qnm")
                    if li == 0:
                        for ch in range(NCH):
                            sl = slice(ch * 128, (ch + 1) * 128)
                            o = qnm[:, ch * 32:(ch + 1) * 32]
                            nc.tensor.matmul(o, nfT[:, sl], w0a[:], start=True, stop=False)
                            nc.tensor.matmul(o, e2nT[:, sl], w0b[:], start=False, stop=True)
                    else:
                        tW, tB, wn = wl[li - 1]
                        pb = 32 * (li - 1)
                        for ch in range(NCH):
                            nc.tensor.matmul(
                                qnm[:, ch * w:(ch + 1) * w],
                                catT[pb:pb + 32, ch * 128:(ch + 1) * 128],
                                tW[pb:pb + 32, :], start=True, stop=True)

                    # -- triple bf16 split, built in 4 kt-groups; narrowing
                    #    copies on ACT, exact f32 residuals on DVE --
                    qsg = []
                    qv = qnm[:].rearrange("p (c j) -> p c j", j=w)
                    for j in range(4):
                        qs = pL.tile([128, 4, 96], dt.bfloat16, tag=f"qs{j}")
                        if w == 1:
                            nc.gpsimd.memset(qs[:], 0.0)
                        src = qv[:, j * 4:(j + 1) * 4, :]
                        t2 = pL.tile([128, 4, w], dt.float32, tag=f"t2{j}")
                        t3 = pL.tile([128, 4, w], dt.float32, tag=f"t3{j}")
                        nc.scalar.activation(qs[:, :, 0:w], src, AF.Copy)
                        nc.vector.tensor_tensor(t2[:], src, qs[:, :, 0:w], op=OP.subtract)
                        nc.scalar.activation(qs[:, :, 32:32 + w], t2[:], AF.Copy)
                        nc.vector.tensor_tensor(t3[:], t2[:], qs[:, :, 32:32 + w], op=OP.subtract)
                        nc.vector.tensor_copy(qs[:, :, 64:64 + w], t3[:])
                        qsg.append(qs)

                    # -- A-pass in four 512-col quarters, kt-outer.  The E3
                    #    partition-sum matmul for quarter q is emitted after
                    #    quarter q+1's matmuls so the PE never waits on the
                    #    ACT psum->sbuf copy; STT fuses bias+invdeg, ACT tanh
                    #    lands rows straight into catT --
                    def emit_quarter_mms(q):
                        pT = psP.tile([96, 512], dt.float32, tag="pT")
                        csl = slice(q * 512, (q + 1) * 512)
                        for kt in range(NCH):
                            nc.tensor.matmul(
                                pT[:], qsg[kt // 4][:, kt % 4, :],
                                at[:, kt, csl],
                                start=(kt == 0), stop=(kt == NCH - 1))
                        pTs = pL.tile([96, 512], dt.float32, tag="pTs")
                        nc.scalar.activation(pTs[:], pT[:], AF.Copy)
                        return pTs

                    def emit_epilogue(q, pTs, li=li, w=w, bias=bias):
                        csl = slice(q * 512, (q + 1) * 512)
                        ps3 = psQ.tile([32, 512], dt.float32, tag="qnm")
                        nc.tensor.matmul(ps3[0:32, :], e3[:, 0:32], pTs[:],
                                         start=True, stop=True)
                        h = pL.tile([w, 512], dt.float32, tag="h")
                        nc.vector.scalar_tensor_tensor(
                            h[:], ps3[0:w, :], bias[:], degb[0:w, csl],
                            op0=OP.add, op1=OP.mult)
                        nc.scalar.activation(
                            catT[32 * li:32 * li + w, csl], h[:], AF.Tanh)

                    prev = emit_quarter_mms(0)
                    for q in range(1, 4):
                        cur = emit_quarter_mms(q)
                        emit_epilogue(q - 1, prev)
                        prev = cur
                    emit_epilogue(3, prev)

                    # interleave the NEXT graph's e2n work and the PREVIOUS
                    # graph's cat transpose-out into this graph's layer gaps
                    # (one 4-chunk group of each per layer)
                    if g + 1 < GPC:
                        emit_e2n_group(g + 1, e2nTs[g + 1], li)
                    if g > 0:
                        emit_cat_group(g - 1, prev_catT, li)

                    if li < 3:
                        bias = wl[li][1]

                # sort channel row for this graph: DMA f-major h4 into sortbuf[g]
                nc.scalar.dma_start(sortbuf[g:g + 1, :], catT[96:97, :])
                prev_catT = catT

            # flush the last graph's cat transpose-out
            for grp in range(NGRP):
                emit_cat_group(GPC - 1, prev_catT, grp)

            # ---- sortpooling: top-30 (+2 spare) per graph ----
            tv = pS.tile([GPC, 32], dt.float32)
            ti = pS.tile([GPC, 32], dt.uint32)
            for r in range(4):
                nc.vector.max(tv[:, r * 8:(r + 1) * 8], sortbuf[:])
                nc.vector.max_index(ti[:, r * 8:(r + 1) * 8], tv[:, r * 8:(r + 1) * 8], sortbuf[:])
                if r < 3:
                    nc.vector.match_replace(sortbuf[:], tv[:, r * 8:(r + 1) * 8], sortbuf[:], -1e30)

            # transpose indices to column-major [32, GPC] for indirect DMA
            tiTu = pS.tile([32, GPC], dt.uint32)
            nc.scalar.dma_start(d_tis.ap(), ti[:])
            nc.scalar.dma_start(tiTu[:], d_tis.ap().rearrange("a b -> b a"))

            # ---- head, batched over the core's graphs ----
            rhs_all = pS.tile([TLD, GPC * K], dt.float32)
            for g in range(GPC):
                pg = pS.tile([K, TLD], dt.float32, tag="pg")
                gi = nc.gpsimd.indirect_dma_start(
                    out=pg[:], out_offset=None, in_=d_cat.ap(),
                    in_offset=bass.IndirectOffsetOnAxis(ap=tiTu[0:K, g:g + 1], axis=0),
                    element_offset=g * NPG * TLD)
                for _cd in cat_dmas:
                    add_dep_helper(gi.ins, _cd, reason="gather after cat writes")
                ppg = psT.tile([TLD, K], dt.float32, tag="tr")
                nc.tensor.transpose(ppg[:], pg[:], ident[0:K, 0:K])
                nc.vector.tensor_copy(rhs_all[:, g * K:(g + 1) * K], ppg[:])

            ps1 = psQ.tile([C1, GPC * K], dt.float32, tag="qnm")
            nc.tensor.matmul(ps1[:], wc1[:], rhs_all[:], start=True, stop=True)
            y1 = pS.tile([C1, GPC * K], dt.float32)
            nc.scalar.activation(y1[:], ps1[:], AF.Relu, bias=bc1[:])
            y1v = y1[:].rearrange("a (g k t) -> a g k t", g=GPC, t=2)
            y2 = pS.tile([C1, GPC, K // 2], dt.float32)
            nc.vector.tensor_tensor(y2[:], y1v[:, :, :, 0], y1v[:, :, :, 1], op=OP.max)

            ps2 = psQ.tile([C2, GPC * 11], dt.float32, tag="qnm")
            r2t = pS.tile([C1, GPC * 11], dt.float32, tag="r2t")
            for t in range(KW2):
                r2tv = r2t[:].rearrange("a (g j) -> a g j", g=GPC)
                nc.vector.tensor_copy(r2tv, y2[:, :, t:t + 11])
                nc.tensor.matmul(ps2[:], wc2[:, t, :], r2t[:], start=(t == 0), stop=(t == KW2 - 1))
            y3 = pS.tile([C2, GPC * 11], dt.float32)
            nc.scalar.activation(y3[:], ps2[:], AF.Relu, bias=bc2[:])
            y3v = y3[:].rearrange("a (g j) -> a g j", g=GPC)

            psh = psQ.tile([HID, GPC], dt.float32, tag="qnm")
            r3t = pS.tile([C2, GPC], dt.float32, tag="r3t")
            for j in range(11):
                nc.vector.tensor_copy(r3t[:], y3v[:, :, j])
                nc.tensor.matmul(psh[:], wh[:, j, :], r3t[:], start=(j == 0), stop=(j == 10))
            h1 = pS.tile([HID, GPC], dt.float32)
            nc.scalar.activation(h1[:], psh[:], AF.Relu, bias=bh[:])

            psl = psQ.tile([NCLS, GPC], dt.float32, tag="qnm")
            nc.tensor.matmul(psl[:], wo[:], h1[:], start=True, stop=True)
            lg0 = pS.tile([NCLS, GPC], dt.float32)
            nc.scalar.activation(lg0[:], psl[:], AF.Identity, bias=bo[:])
            plT = psT.tile([GPC, NCLS], dt.float32, tag="tr")
            nc.tensor.transpose(plT[:], lg0[:], ident[0:NCLS, 0:NCLS])
            lgT = pS.tile([GPC, NCLS], dt.float32)
            nc.vector.tensor_copy(lgT[:], plT[:])

            mx = pS.tile([GPC, 1], dt.float32)
            nc.vector.tensor_reduce(mx[:], lgT[:], axis=AX.X, op=OP.max)
            sh = pS.tile([GPC, NCLS], dt.float32)
            nc.vector.tensor_scalar(sh[:], lgT[:], mx[:], None, op0=OP.subtract)
            ex = pS.tile([GPC, NCLS], dt.float32)
            sm = pS.tile([GPC, 1], dt.float32)
            nc.scalar.activation(ex[:], sh[:], AF.Exp, accum_out=sm[:])
            lsm = pS.tile([GPC, 1], dt.float32)
            nc.scalar.activation(lsm[:], sm[:], AF.Ln)
            osm = pS.tile([GPC, NCLS], dt.float32)
            nc.vector.tensor_scalar(osm[:], sh[:], lsm[:], None, op0=OP.subtract)
            nc.scalar.dma_start(d_out.ap(), osm[:])

    nc.compile()
    return nc


def _prep_inputs(node_feat, edge_feat, node_degs, W0, b0, W1, b1, W2, b2, W3, b3,
                 Wc1, bc1, Wc2, bc2, Wh, bh, Wo, bo, edge_row, edge_col):
    """Host-side index preprocessing + per-core input maps."""
    bf8 = ml_dtypes.float8_e4m3

    deg_all = np.bincount(edge_row, minlength=N)
    maxdeg = int(deg_all.max())
    S = max(32, ((maxdeg + 7) // 8) * 8)

    # shared weight tensors
    shared = {
        "w0aT": np.ascontiguousarray(W0[:, :NF].T),
        "w0bT": np.ascontiguousarray(W0[:, NF:].T),
        "w1T3": np.tile(np.ascontiguousarray(W1.T), (3, 1)),
        "w2T3": np.tile(np.ascontiguousarray(W2.T), (3, 1)),
        "w3T3": np.tile(np.ascontiguousarray(W3.T), (3, 1)),
        "b0c": b0.reshape(32, 1), "b1c": b1.reshape(32, 1),
        "b2c": b2.reshape(32, 1), "b3c": b3.reshape(1, 1),
        "wc1T": np.ascontiguousarray(Wc1.T),
        "wc2T": np.ascontiguousarray(Wc2.transpose(1, 2, 0)),  # [C1, KW2, C2]
        "bc1c": bc1.reshape(C1, 1), "bc2c": bc2.reshape(C2, 1),
        "whT": np.ascontiguousarray(Wh.reshape(HID, C2, 11).transpose(1, 2, 0)),  # [C2, 11, HID]
        "bhc": bh.reshape(HID, 1),
        "woT": np.ascontiguousarray(Wo.T), "boc": bo.reshape(NCLS, 1),
        "e3c": np.tile(np.eye(32, dtype=np.float32), (3, 1)),
    }
    shared = {k: v.astype(np.float32) for k, v in shared.items()}

    in_maps = []
    for c in range(NCORES):
        gs = range(c * GPC, (c + 1) * GPC)
        nfT = np.empty((GPC, NF, NPG), np.float32)
        efp = np.zeros((GPC, NGRP, 128, 4, EF, S), np.float32)
        at8 = np.empty((GPC, 128, NCH, NPG), bf8)
        degb = np.empty((GPC, 32, NPG), np.float32)
        for i, g in enumerate(gs):
            nsl = slice(g * NPG, (g + 1) * NPG)
            esl = slice(g * EPG, (g + 1) * EPG)
            er = edge_row[esl] - g * NPG
            ec = edge_col[esl] - g * NPG
            nfT[i] = node_feat[nsl].T
            degb[i] = np.broadcast_to((np.float32(1.0) / node_degs[nsl].astype(np.float32)).reshape(1, NPG), (32, NPG))
            # A_aug^T: [m, n] = count(col=m, row=n) + I
            cnt = np.bincount(ec.astype(np.int64) * NPG + er, minlength=NPG * NPG)
            A = cnt.reshape(NPG, NPG).astype(np.float32)
            A[np.arange(NPG), np.arange(NPG)] += 1.0
            assert A.max() <= 16, "fp8 e4m3 exact-count range exceeded"
            at8[i] = A.reshape(NCH, 128, NPG).transpose(1, 0, 2).astype(bf8)
            # padded row-sorted edge features: node-major [node, feat, slot],
            # regrouped as [grp, p, c, feat, slot] with node = (4*grp + c)*128 + p
            order = np.argsort(er, kind="stable")
            sr = er[order]
            deg = np.bincount(sr, minlength=NPG)
            starts = np.zeros(NPG, np.int64)
            starts[1:] = np.cumsum(deg)[:-1]
            pos = np.arange(EPG) - starts[sr]
            ef_g = edge_feat[esl][order]
            dst = np.zeros((NPG, EF, S), np.float32)
            dst[sr, :, pos] = ef_g
            efp[i] = dst.reshape(NGRP, 4, 128, EF, S).transpose(0, 2, 1, 3, 4)
        m = dict(shared)
        m.update(nfT=nfT, efp=efp, at8=at8, degb=degb)
        in_maps.append(m)
    return in_maps, S


def kernel(**inputs):
    from concourse.bass_utils import run_bass_kernel_spmd
    in_maps, S = _prep_inputs(**inputs)
    if ("nc", S) not in _cache:
        _cache[("nc", S)] = _build_program(S)
    nc = _cache[("nc", S)]
    res = run_bass_kernel_spmd(nc, in_maps, core_ids=list(range(NCORES)))
    out = np.concatenate([res.results[c]["out"] for c in range(NCORES)], axis=0)
    return out.astype(np.float32)
